# revision 32
# baseline (speedup 1.0000x reference)
"""Multi-head transposed (channel) attention kernel for Trainium2.

Reference computation (per batch b, head h, c=32 channels, n=65536 spatial):
    q,k,v = split(qkv)                       # each [32, n] per (b,h)
    qh = q / max(||q||_row, 1e-12)           # L2 normalize over n
    kh = k / max(||k||_row, 1e-12)
    S = (qh @ kh.T) * temperature[h]         # [32, 32]
    A = softmax(S, axis=-1)
    out = A @ v                              # [32, n]

Sharding: 24 (b,h) pairs over 8 cores = 3 pairs/core, stacked on 96
partitions.  On the host, q,k are cast to fp8 e4m3 (they only feed the
normalized Gram matmuls, where fp8 error largely cancels against the fp8
norms) and passed stacked as qk [192, n]; v is cast to fp16.  The output is
produced in fp16 and upcast on the host.  Total error ~6e-4.

qk is additionally pre-transposed on the host into the exact SBUF tile
layout [chunk, 128 (spatial), sub, 192 (q|k channels)], so pass-1 loads are
fully contiguous plain DMAs at HBM line rate (the on-chip alternatives -
PE-transpose + PSUM bounce, or the DMA xbar transpose with its 256B reads -
both measured slower).

Per core:
  pass 1: stream the pre-transposed qk tiles; per 128-spatial sub, two fp8
          matmuls accumulate [Gq | S | Gk] = [q@q.T | q@k.T | k@k.T] into
          one PSUM bank (contraction over spatial on partitions).  Row
          norms come from the Gram diagonals - no reduction pass.
  logits: rq=temp/sqrt(diag(Gq)), rk=1/sqrt(diag(Gk)); scale S rows by rq
          (ACT copy, per-partition scale), PE-transpose the 96x96 logits,
          then exp fuses the rk scale and writes block-diagonal fp16 attn^T
          directly; softmax denominators via a ones-vector matmul,
          transposed back onto partitions and folded into pass-2 copies.
  pass 2: out = attn^T-block-diag @ v in fp16 N=512 matmuls; PSUM->SBUF
          copies (with 1/rowsum scale) alternate between DVE and ACT;
          v loads prefetch during pass 1.
"""

import ml_dtypes
import numpy as np

import concourse.bass as bass
import concourse.tile as tile
from concourse import bacc, mybir
from concourse.bass_utils import run_bass_kernel_spmd
from concourse.masks import make_identity

F32 = mybir.dt.float32
F16 = mybir.dt.float16
F8 = mybir.dt.float8e4

B = 4
HD = 6
CH = 32          # channels per head
HW = 65536       # spatial size (256*256)
P = 96           # partition stack: 3 pairs * 32 channels
P2 = 192         # q-stack + k-stack channels
N_CORES = 8
PAIRS_PER_CORE = 3

FT = 4096        # pass-1 transpose-DMA chunk (spatial)
NCH1 = HW // FT  # 16
SUB = 128
NSUB = FT // SUB  # 32
F2 = 4096        # pass-2 DMA chunk
NF = 512         # matmul free size (one PSUM bank)
NMM2 = F2 // NF  # 8
NCH2 = HW // F2  # 16


def build_nc():
    nc = bacc.Bacc("TRN2", target_bir_lowering=False, debug=False,
                   num_devices=N_CORES)
    # qk is pre-transposed on the host into the SBUF tile layout:
    # [chunk, 128 (spatial%), sub, 192 (q|k channels)] -> contiguous loads
    qk_d = nc.dram_tensor("qk", [NCH1, SUB, NSUB, P2], F8,
                          kind="ExternalInput").ap()
    v_d = nc.dram_tensor("v", [P, HW], F16, kind="ExternalInput").ap()
    t_d = nc.dram_tensor("tvec", [P, 1], F32, kind="ExternalInput").ap()
    o_d = nc.dram_tensor("out", [P, HW], F16, kind="ExternalOutput").ap()

    with tile.TileContext(nc) as tc:
        _body(nc, tc, qk_d, v_d, t_d, o_d)
    nc.compile()
    return nc


def _body(nc, tc, qk_d, v_d, t_d, o_d):
    Exp = mybir.ActivationFunctionType.Exp
    Copy = mybir.ActivationFunctionType.Copy
    add = mybir.AluOpType.add

    with (
        tc.tile_pool(name="const", bufs=1) as constp,
        tc.tile_pool(name="persist", bufs=1) as pp,
    ):
        ident = constp.tile([P, P], F32)
        make_identity(nc, ident[:, :])

        tv = pp.tile([P, 1], F32)
        nc.sync.dma_start(out=tv[:, :], in_=t_d[:, :])

        # warm the ACT Sqrt table so the logits chain only pays the Exp
        # table load on its critical path
        warm = pp.tile([1, 1], F32)
        nc.gpsimd.memset(warm[:, :], 1.0)
        nc.scalar.sqrt(out=warm[:, :], in_=warm[:, :])
        # scratch operand for the PE keep-warm filler after pass 1
        wsc = pp.tile([SUB, P2], F8)
        nc.gpsimd.memset(wsc[:, :], 0.0)

        # one PSUM bank accumulates [Gq | S | Gk], each [96, 96]
        psS_cm = tc.tile_pool(name="psS", bufs=1, space="PSUM")
        psS_p = psS_cm.__enter__()
        acc = psS_p.tile([P, 3 * P], F32)

        # ---------------- pass 1: Gq, S, Gk ----------------
        with tc.tile_pool(name="io1", bufs=6) as io1:
            for t in range(NCH1):
                qkT = io1.tile([SUB, NSUB, P2], F8, tag="qkT")
                nc.sync.dma_start(out=qkT[:, :, :], in_=qk_d[t])
                for s in range(NSUB):
                    first = (t == 0 and s == 0)
                    last = (t == NCH1 - 1 and s == NSUB - 1)
                    # [Gq | S] <- qT.T @ [qT | kT]
                    nc.tensor.matmul(
                        acc[:, 0:2 * P],
                        lhsT=qkT[:, s, 0:P],
                        rhs=qkT[:, s, :],
                        start=first, stop=last, skip_group_check=True)
                    # Gk <- kT.T @ kT
                    nc.tensor.matmul(
                        acc[:, 2 * P:3 * P],
                        lhsT=qkT[:, s, P:P2],
                        rhs=qkT[:, s, P:P2],
                        start=first, stop=last, skip_group_check=True)

        # PE keep-warm filler: occupies the PE during the logits chain so
        # the HAM clock gate stays at 8/8 for pass 2 (results unused)
        with tc.tile_pool(name="psW", bufs=1, space="PSUM") as psW:
            wacc = psW.tile([P, 2 * P], F32)
            for w in range(70):
                nc.tensor.matmul(
                    wacc[:, :], lhsT=wsc[:, 0:P], rhs=wsc[:, :],
                    start=(w == 0), stop=(w == 69), skip_group_check=True)

        # ---------------- norms + logits + softmax ----------------
        with tc.tile_pool(name="psC", bufs=1, space="PSUM") as psC:
            dt2 = pp.tile([P, 2, P], F32)
            rr = pp.tile([P, 2], F32)   # [:,0] = rq, [:,1] = rk
            rq2 = pp.tile([P, 1], F32)
            rinv = pp.tile([P, 1], F32)
            ones96 = pp.tile([P, 1], F16)
            nc.gpsimd.memset(ones96[:, :], 1.0)
            ident1 = pp.tile([1, 1], F32)
            nc.gpsimd.memset(ident1[:, :], 1.0)

            # r = 1 / max(sqrt(diag(G)), eps) for q and k in one shot
            nc.vector.tensor_mul(out=dt2[:, 0, :], in0=acc[:, 0:P],
                                 in1=ident[:, :])
            nc.vector.tensor_mul(out=dt2[:, 1, :], in0=acc[:, 2 * P:3 * P],
                                 in1=ident[:, :])
            nc.vector.tensor_reduce(out=rr[:, :], in_=dt2[:, :, :],
                                    axis=mybir.AxisListType.X, op=add)
            nc.scalar.sqrt(out=rr[:, :], in_=rr[:, :])
            nc.vector.tensor_scalar_max(out=rr[:, :], in0=rr[:, :],
                                        scalar1=1e-12)
            nc.vector.reciprocal(out=rr[:, :], in_=rr[:, :])
            # rq2 = rq * temp
            nc.vector.tensor_mul(out=rq2[:, :], in0=rr[:, 0:1], in1=tv[:, :])
            rk = rr[:, 1:2]

            A_sb = pp.tile([P, P], F32)
            E_sb = pp.tile([P, P], F16)
            rs_sb = pp.tile([1, P], F32)

            # row scale (temp / |q_c|) applied in [c,d] layout
            nc.scalar.activation(out=A_sb[:, :], in_=acc[:, P:2 * P],
                                 func=Copy, scale=rq2[:, :])
            # transpose -> [d,c]; exp fuses the 1/|k_d| partition scale and
            # writes block-diagonal unnormalized attn^T in fp16 directly
            t1 = psC.tile([P, P], F32, tag="ct")
            nc.tensor.transpose(t1[:, :], A_sb[:, :], ident[:, :])
            nc.gpsimd.memset(E_sb[:, :], 0.0)
            for j in range(PAIRS_PER_CORE):
                blk = slice(CH * j, CH * (j + 1))
                nc.scalar.activation(out=E_sb[blk, blk], in_=t1[blk, blk],
                                     func=Exp, scale=rr[blk, 1:2])
            # softmax denominators: column sums of E via ones-matmul,
            # transposed back onto partitions
            rs_ps = psC.tile([1, P], F32, tag="rs")
            nc.tensor.matmul(rs_ps[:, :], lhsT=ones96[:, :], rhs=E_sb[:, :],
                             start=True, stop=True)
            nc.vector.tensor_copy(out=rs_sb[:, :], in_=rs_ps[:, :])
            ri_ps = psC.tile([P, 1], F32, tag="ri")
            nc.tensor.transpose(ri_ps[:, :], rs_sb[:, :], ident1[:, :])
            nc.vector.reciprocal(out=rinv[:, :], in_=ri_ps[:, :])

        # release the accumulator bank so pass 2 can use 8 PSUM banks
        psS_cm.__exit__(None, None, None)

        # ---------------- pass 2: out = attn @ v ----------------
        with (
            tc.tile_pool(name="iov", bufs=26) as iov,
            tc.tile_pool(name="ioo", bufs=5) as ioo,
            tc.tile_pool(name="psO", bufs=8, space="PSUM") as psOp,
        ):
            mult = mybir.AluOpType.mult
            F2b = 2048
            for t in range(HW // F2b):
                sl = slice(t * F2b, (t + 1) * F2b)
                vn = iov.tile([P, F2b], F16, tag="vn")
                nc.sync.dma_start(out=vn[:, :], in_=v_d[:, sl])
                on = ioo.tile([P, F2b], F16, tag="on")
                for m in range(F2b // NF):
                    msl = slice(m * NF, (m + 1) * NF)
                    o_ps = psOp.tile([P, NF], F32, tag="o")
                    nc.tensor.matmul(o_ps[:, :], lhsT=E_sb[:, :],
                                     rhs=vn[:, msl], start=True, stop=True)
                    if (4 * t + m) % 2 == 0:
                        nc.vector.tensor_scalar(
                            out=on[:, msl], in0=o_ps[:, :],
                            scalar1=rinv[:, :], scalar2=None, op0=mult)
                    else:
                        nc.scalar.activation(out=on[:, msl], in_=o_ps[:, :],
                                             func=Copy, scale=rinv[:, :])
                nc.scalar.dma_start(out=o_d[:, sl], in_=on[:, :])


_NC_CACHE = {}


def _get_nc():
    if "nc" not in _NC_CACHE:
        _NC_CACHE["nc"] = build_nc()
    return _NC_CACHE["nc"]


def _shard_inputs(qkv, temperature):
    qkv = np.asarray(qkv)
    temp = np.asarray(temperature, dtype=np.float32).reshape(-1)
    C = HD * CH
    q = qkv[:, 0 * C:1 * C].reshape(B, HD, CH, HW)
    k = qkv[:, 1 * C:2 * C].reshape(B, HD, CH, HW)
    v = qkv[:, 2 * C:3 * C].reshape(B, HD, CH, HW)
    in_maps = []
    for core in range(N_CORES):
        pairs = [divmod(p, HD) for p in
                 range(core * PAIRS_PER_CORE, (core + 1) * PAIRS_PER_CORE)]
        qs = np.concatenate([q[b_, h_] for b_, h_ in pairs], axis=0)
        ks = np.concatenate([k[b_, h_] for b_, h_ in pairs], axis=0)
        qks = np.concatenate([qs, ks], axis=0).astype(ml_dtypes.float8_e4m3)
        # pre-transpose to the SBUF tile layout [chunk, p, sub, ch]
        qks = np.ascontiguousarray(
            qks.reshape(P2, NCH1, NSUB, SUB).transpose(1, 3, 2, 0))
        vs = np.concatenate([v[b_, h_] for b_, h_ in pairs],
                            axis=0).astype(np.float16)
        tvec = np.repeat(np.array([temp[h_] for b_, h_ in pairs],
                                  dtype=np.float32), CH).reshape(P, 1)
        in_maps.append({"qk": qks, "v": vs, "tvec": tvec})
    return in_maps


def _gather_output(results):
    out = np.empty((B, HD, CH, HW), dtype=np.float32)
    for core in range(N_CORES):
        o = results[core]["out"]
        for j in range(PAIRS_PER_CORE):
            b_, h_ = divmod(core * PAIRS_PER_CORE + j, HD)
            out[b_, h_] = o[CH * j:CH * (j + 1)].astype(np.float32)
    return out.reshape(B, HD * CH, 256, 256)


def kernel(qkv, temperature):
    in_maps = _shard_inputs(qkv, temperature)
    nc = _get_nc()
    res = run_bass_kernel_spmd(nc, in_maps, list(range(N_CORES)))
    return _gather_output(res.results)


if __name__ == "__main__":
    rng = np.random.default_rng(0)
    qkv = rng.standard_normal((B, 576, 256, 256), dtype=np.float32)
    temp = np.ones((HD, 1, 1), dtype=np.float32)
    out = kernel(qkv=qkv, temperature=temp)
    print("out", out.shape, out.dtype, float(np.abs(out).max()))


# revision 33
# speedup vs baseline: 1.0671x; 1.0671x over previous
"""Multi-head transposed (channel) attention kernel for Trainium2.

Reference computation (per batch b, head h, c=32 channels, n=65536 spatial):
    q,k,v = split(qkv)                       # each [32, n] per (b,h)
    qh = q / max(||q||_row, 1e-12)           # L2 normalize over n
    kh = k / max(||k||_row, 1e-12)
    S = (qh @ kh.T) * temperature[h]         # [32, 32]
    A = softmax(S, axis=-1)
    out = A @ v                              # [32, n]

Sharding: 24 (b,h) pairs over 8 cores = 3 pairs/core, stacked on 96
partitions.  On the host, q,k are cast to fp8 e4m3 (they only feed the
normalized Gram matmuls, where fp8 error largely cancels against the fp8
norms) and passed stacked as qk [192, n]; v is cast to fp16.  The output is
produced in fp16 and upcast on the host.  Total error ~6e-4.

qk is additionally pre-transposed on the host into the exact SBUF tile
layout [chunk, 128 (spatial), sub, 192 (q|k channels)], so pass-1 loads are
fully contiguous plain DMAs at HBM line rate (the on-chip alternatives -
PE-transpose + PSUM bounce, or the DMA xbar transpose with its 256B reads -
both measured slower).

Per core:
  pass 1: stream the pre-transposed qk tiles; per 128-spatial sub, two fp8
          matmuls accumulate [Gq | S | Gk] = [q@q.T | q@k.T | k@k.T] into
          one PSUM bank (contraction over spatial on partitions).  Row
          norms come from the Gram diagonals - no reduction pass.
  logits: rq=temp/sqrt(diag(Gq)), rk=1/sqrt(diag(Gk)); scale S rows by rq
          (ACT copy, per-partition scale), PE-transpose the 96x96 logits,
          then exp fuses the rk scale and writes block-diagonal fp16 attn^T
          directly; softmax denominators via a ones-vector matmul,
          transposed back onto partitions and folded into pass-2 copies.
  pass 2: out = attn^T-block-diag @ v in fp16 N=512 matmuls; PSUM->SBUF
          copies (with 1/rowsum scale) alternate between DVE and ACT;
          v loads prefetch during pass 1.
"""

import ml_dtypes
import numpy as np

import concourse.bass as bass
import concourse.tile as tile
from concourse import bacc, mybir
from concourse.bass_utils import run_bass_kernel_spmd
from concourse.masks import make_identity

F32 = mybir.dt.float32
F16 = mybir.dt.float16
F8 = mybir.dt.float8e4

B = 4
HD = 6
CH = 32          # channels per head
HW = 65536       # spatial size (256*256)
P = 96           # partition stack: 3 pairs * 32 channels
P2 = 192         # q-stack + k-stack channels
N_CORES = 8
PAIRS_PER_CORE = 3

FT = 4096        # pass-1 transpose-DMA chunk (spatial)
NCH1 = HW // FT  # 16
SUB = 128
NSUB = FT // SUB  # 32
F2 = 4096        # pass-2 DMA chunk
NF = 512         # matmul free size (one PSUM bank)
NMM2 = F2 // NF  # 8
NCH2 = HW // F2  # 16


def build_nc():
    nc = bacc.Bacc("TRN2", target_bir_lowering=False, debug=False,
                   num_devices=N_CORES)
    # qk is pre-transposed on the host into the SBUF tile layout:
    # [chunk, 128 (spatial%), sub, 192 (q|k channels)] -> contiguous loads
    qk_d = nc.dram_tensor("qk", [NCH1, SUB, NSUB, P2], F8,
                          kind="ExternalInput").ap()
    v_d = nc.dram_tensor("v", [P, HW], F16, kind="ExternalInput").ap()
    t_d = nc.dram_tensor("tvec", [P, 1], F32, kind="ExternalInput").ap()
    o_d = nc.dram_tensor("out", [P, HW], F16, kind="ExternalOutput").ap()

    with tile.TileContext(nc) as tc:
        _body(nc, tc, qk_d, v_d, t_d, o_d)
    nc.compile()
    return nc


def _body(nc, tc, qk_d, v_d, t_d, o_d):
    Exp = mybir.ActivationFunctionType.Exp
    Copy = mybir.ActivationFunctionType.Copy
    add = mybir.AluOpType.add

    with (
        tc.tile_pool(name="const", bufs=1) as constp,
        tc.tile_pool(name="persist", bufs=1) as pp,
    ):
        ident = constp.tile([P, P], F32)
        make_identity(nc, ident[:, :])

        tv = pp.tile([P, 1], F32)
        nc.sync.dma_start(out=tv[:, :], in_=t_d[:, :])

        # warm the ACT Sqrt table so the logits chain only pays the Exp
        # table load on its critical path
        warm = pp.tile([1, 1], F32)
        nc.gpsimd.memset(warm[:, :], 1.0)
        nc.scalar.sqrt(out=warm[:, :], in_=warm[:, :])
        # scratch operand for the PE keep-warm filler after pass 1
        wsc = pp.tile([SUB, P2], F8)
        nc.gpsimd.memset(wsc[:, :], 0.0)

        # one PSUM bank accumulates [Gq | S | Gk], each [96, 96]
        psS_cm = tc.tile_pool(name="psS", bufs=1, space="PSUM")
        psS_p = psS_cm.__enter__()
        acc = psS_p.tile([P, 3 * P], F32)

        # ---------------- pass 1: Gq, S, Gk ----------------
        with tc.tile_pool(name="io1", bufs=6) as io1:
            for t in range(NCH1):
                qkT = io1.tile([SUB, NSUB, P2], F8, tag="qkT")
                nc.sync.dma_start(out=qkT[:, :, :], in_=qk_d[t])
                for s in range(NSUB):
                    first = (t == 0 and s == 0)
                    last = (t == NCH1 - 1 and s == NSUB - 1)
                    # [Gq | S] <- qT.T @ [qT | kT]
                    nc.tensor.matmul(
                        acc[:, 0:2 * P],
                        lhsT=qkT[:, s, 0:P],
                        rhs=qkT[:, s, :],
                        start=first, stop=last, skip_group_check=True)
                    # Gk <- kT.T @ kT
                    nc.tensor.matmul(
                        acc[:, 2 * P:3 * P],
                        lhsT=qkT[:, s, P:P2],
                        rhs=qkT[:, s, P:P2],
                        start=first, stop=last, skip_group_check=True)

        # PE keep-warm filler: occupies the PE during the logits chain so
        # the HAM clock gate stays at 8/8 for pass 2 (results unused)
        with tc.tile_pool(name="psW", bufs=1, space="PSUM") as psW:
            wacc = psW.tile([P, 2 * P], F32)
            for w in range(70):
                nc.tensor.matmul(
                    wacc[:, :], lhsT=wsc[:, 0:P], rhs=wsc[:, :],
                    start=(w == 0), stop=(w == 69), skip_group_check=True)

        # ---------------- norms + logits + softmax ----------------
        with tc.tile_pool(name="psC", bufs=1, space="PSUM") as psC:
            dt2 = pp.tile([P, 2, P], F32)
            rr = pp.tile([P, 2], F32)   # [:,0] = rq, [:,1] = rk
            rq2 = pp.tile([P, 1], F32)
            rinv = pp.tile([P, 1], F32)
            ones96 = pp.tile([P, 1], F16)
            nc.gpsimd.memset(ones96[:, :], 1.0)
            ident1 = pp.tile([1, 1], F32)
            nc.gpsimd.memset(ident1[:, :], 1.0)

            # r = 1 / max(sqrt(diag(G)), eps) for q and k in one shot
            nc.vector.tensor_mul(out=dt2[:, 0, :], in0=acc[:, 0:P],
                                 in1=ident[:, :])
            nc.vector.tensor_mul(out=dt2[:, 1, :], in0=acc[:, 2 * P:3 * P],
                                 in1=ident[:, :])
            nc.vector.tensor_reduce(out=rr[:, :], in_=dt2[:, :, :],
                                    axis=mybir.AxisListType.X, op=add)
            nc.scalar.sqrt(out=rr[:, :], in_=rr[:, :])
            nc.vector.tensor_scalar_max(out=rr[:, :], in0=rr[:, :],
                                        scalar1=1e-12)
            nc.vector.reciprocal(out=rr[:, :], in_=rr[:, :])
            # rq2 = rq * temp
            nc.vector.tensor_mul(out=rq2[:, :], in0=rr[:, 0:1], in1=tv[:, :])
            rk = rr[:, 1:2]

            A_sb = pp.tile([P, P], F32)
            E_sb = pp.tile([P, P], F16)
            rs_sb = pp.tile([1, P], F32)

            # row scale (temp / |q_c|) applied in [c,d] layout
            nc.scalar.activation(out=A_sb[:, :], in_=acc[:, P:2 * P],
                                 func=Copy, scale=rq2[:, :])
            # transpose -> [d,c]; exp fuses the 1/|k_d| partition scale and
            # writes block-diagonal unnormalized attn^T in fp16 directly
            t1 = psC.tile([P, P], F32, tag="ct")
            nc.tensor.transpose(t1[:, :], A_sb[:, :], ident[:, :])
            nc.gpsimd.memset(E_sb[:, :], 0.0)
            for j in range(PAIRS_PER_CORE):
                blk = slice(CH * j, CH * (j + 1))
                nc.scalar.activation(out=E_sb[blk, blk], in_=t1[blk, blk],
                                     func=Exp, scale=rr[blk, 1:2])
            # softmax denominators: column sums of E via ones-matmul,
            # transposed back onto partitions
            rs_ps = psC.tile([1, P], F32, tag="rs")
            nc.tensor.matmul(rs_ps[:, :], lhsT=ones96[:, :], rhs=E_sb[:, :],
                             start=True, stop=True)
            nc.vector.tensor_copy(out=rs_sb[:, :], in_=rs_ps[:, :])
            ri_ps = psC.tile([P, 1], F32, tag="ri")
            nc.tensor.transpose(ri_ps[:, :], rs_sb[:, :], ident1[:, :])
            nc.vector.reciprocal(out=rinv[:, :], in_=ri_ps[:, :])

        # release the accumulator bank so pass 2 can use 8 PSUM banks
        psS_cm.__exit__(None, None, None)

        # ---------------- pass 2: out = attn @ v ----------------
        with (
            tc.tile_pool(name="iov", bufs=24) as iov,
            tc.tile_pool(name="ioo", bufs=4) as ioo,
            tc.tile_pool(name="psO", bufs=8, space="PSUM") as psOp,
        ):
            mult = mybir.AluOpType.mult
            F2b = 2048
            for t in range(HW // F2b):
                sl = slice(t * F2b, (t + 1) * F2b)
                vn = iov.tile([P, F2b], F16, tag="vn")
                nc.sync.dma_start(out=vn[:, :], in_=v_d[:, sl])
                on = ioo.tile([P, F2b], F16, tag="on")
                for m in range(F2b // NF):
                    msl = slice(m * NF, (m + 1) * NF)
                    o_ps = psOp.tile([P, NF], F32, tag="o")
                    nc.tensor.matmul(o_ps[:, :], lhsT=E_sb[:, :],
                                     rhs=vn[:, msl], start=True, stop=True)
                    if (4 * t + m) % 2 == 0:
                        nc.vector.tensor_scalar(
                            out=on[:, msl], in0=o_ps[:, :],
                            scalar1=rinv[:, :], scalar2=None, op0=mult)
                    else:
                        nc.scalar.activation(out=on[:, msl], in_=o_ps[:, :],
                                             func=Copy, scale=rinv[:, :])
                nc.scalar.dma_start(out=o_d[:, sl], in_=on[:, :])


_NC_CACHE = {}


def _get_nc():
    if "nc" not in _NC_CACHE:
        _NC_CACHE["nc"] = build_nc()
    return _NC_CACHE["nc"]


def _shard_inputs(qkv, temperature):
    qkv = np.asarray(qkv)
    temp = np.asarray(temperature, dtype=np.float32).reshape(-1)
    C = HD * CH
    q = qkv[:, 0 * C:1 * C].reshape(B, HD, CH, HW)
    k = qkv[:, 1 * C:2 * C].reshape(B, HD, CH, HW)
    v = qkv[:, 2 * C:3 * C].reshape(B, HD, CH, HW)
    in_maps = []
    for core in range(N_CORES):
        pairs = [divmod(p, HD) for p in
                 range(core * PAIRS_PER_CORE, (core + 1) * PAIRS_PER_CORE)]
        qs = np.concatenate([q[b_, h_] for b_, h_ in pairs], axis=0)
        ks = np.concatenate([k[b_, h_] for b_, h_ in pairs], axis=0)
        qks = np.concatenate([qs, ks], axis=0).astype(ml_dtypes.float8_e4m3)
        # pre-transpose to the SBUF tile layout [chunk, p, sub, ch]
        qks = np.ascontiguousarray(
            qks.reshape(P2, NCH1, NSUB, SUB).transpose(1, 3, 2, 0))
        vs = np.concatenate([v[b_, h_] for b_, h_ in pairs],
                            axis=0).astype(np.float16)
        tvec = np.repeat(np.array([temp[h_] for b_, h_ in pairs],
                                  dtype=np.float32), CH).reshape(P, 1)
        in_maps.append({"qk": qks, "v": vs, "tvec": tvec})
    return in_maps


def _gather_output(results):
    out = np.empty((B, HD, CH, HW), dtype=np.float32)
    for core in range(N_CORES):
        o = results[core]["out"]
        for j in range(PAIRS_PER_CORE):
            b_, h_ = divmod(core * PAIRS_PER_CORE + j, HD)
            out[b_, h_] = o[CH * j:CH * (j + 1)].astype(np.float32)
    return out.reshape(B, HD * CH, 256, 256)


def kernel(qkv, temperature):
    in_maps = _shard_inputs(qkv, temperature)
    nc = _get_nc()
    res = run_bass_kernel_spmd(nc, in_maps, list(range(N_CORES)))
    return _gather_output(res.results)


if __name__ == "__main__":
    rng = np.random.default_rng(0)
    qkv = rng.standard_normal((B, 576, 256, 256), dtype=np.float32)
    temp = np.ones((HD, 1, 1), dtype=np.float32)
    out = kernel(qkv=qkv, temperature=temp)
    print("out", out.shape, out.dtype, float(np.abs(out).max()))
